# revision 16
# baseline (speedup 1.0000x reference)
"""ESM2 backbone (12-layer dense transformer, D=480, H=20, F=1920, L=512)
on 8 Trainium2 NeuronCores.

Sharding: every core carries the full residual stream h (replicated via a
per-layer bf16 delta AllGather) and redundantly computes LN1 / K / V for all
512 tokens; queries, attention softmax/probs, O-projection and the FFN are
sharded over tokens (64 per core).  The attention-map output is sharded over
the query dim; each core writes its own [64,12,20,512] bf16 scratch which the
host reassembles into the final [1,512,512,240] fp32 tensor.

Layout note: every tensor that feeds a bf16 DMA-transpose or a matmul
contraction is padded from 480 to 512 (8 zero rows/cols inserted after every
120) so the xbar 16x128 tile constraint and full-K matmul chunks line up.

Compute dtype: bf16 matmul operands, fp32 PSUM accumulation, fp32 residual.
"""

import os
import sys

sys.path.insert(0, "/opt/trn_rl_repo")

import numpy as np
import ml_dtypes

import concourse.bass as bass
import concourse.bacc as bacc
import concourse.mybir as mybir
import concourse.tile as tile
from concourse.bass_utils import run_bass_kernel_spmd

BF = ml_dtypes.bfloat16
F32 = mybir.dt.float32
BF16 = mybir.dt.bfloat16
AF = mybir.ActivationFunctionType
ALU = mybir.AluOpType

N_CORES = 8
L, D, H, DH, F, NL = 512, 480, 20, 24, 1920, 12
EPS = 1e-5
TOK = L // N_CORES          # 64 own tokens per core
DP = 512                    # padded feature dim (4 chunks: 120 real + 8 pad)
CP = 640                    # padded q/k head dim (5 chunks: 4 heads x 32 rows)
NTC = L // 128              # 4 token chunks of 128

NL_RUN = int(os.environ.get("ESM_NL_RUN", str(NL)))

LAST_EXEC_NS = None
_CACHED = {}


def _real_cols(ap2, r=120, c=128):
    """[P, n*c] -> [P, n, r] AP of the real (non-pad) columns."""
    return ap2.rearrange("p (cc r) -> p cc r", r=c)[:, :, 0:r]


def _halves(ap2, ncc):
    """[P, ncc*128] head-slot padded -> (halvesA, halvesB) APs [P, ncc, 4, 12]."""
    v = ap2.rearrange("p (cc h s) -> p cc h s", s=32, h=4)
    return v[:, :, :, 0:12], v[:, :, :, 12:24]


def build_nc():
    nc = bacc.Bacc("TRN2", target_bir_lowering=False, debug=False,
                   num_devices=N_CORES)

    eps_t = nc.alloc_sbuf_tensor("const-eps", [128, 1], F32)
    nc.gpsimd.memset(eps_t.ap(), float(EPS))
    nc.const_aps.aps[(F32, float(EPS))] = eps_t.ap()
    nc.all_engine_barrier()

    # ---- DRAM I/O ----
    h0_d = nc.dram_tensor("h0", (128, NTC, D), F32, kind="ExternalInput")
    cos_d = nc.dram_tensor("cos_t", (128, NTC, CP), BF16, kind="ExternalInput")
    sina_d = nc.dram_tensor("sina_t", (128, NTC, 240), BF16, kind="ExternalInput")
    sinb_d = nc.dram_tensor("sinb_t", (128, NTC, 240), BF16, kind="ExternalInput")
    h0f_d = nc.dram_tensor("h0f", (L, D), F32, kind="ExternalInput")
    cosf_d = nc.dram_tensor("cosf", (L, CP), BF16, kind="ExternalInput")
    sinaf_d = nc.dram_tensor("sinaf", (L, 240), BF16, kind="ExternalInput")
    sinbf_d = nc.dram_tensor("sinbf", (L, 240), BF16, kind="ExternalInput")
    w_d = {}
    for l in range(NL_RUN):
        w_d[l] = {
            "wq": nc.dram_tensor(f"wq{l}", (128, 4, CP), BF16, kind="ExternalInput"),
            "wk": nc.dram_tensor(f"wk{l}", (128, 4, CP), BF16, kind="ExternalInput"),
            "wv": nc.dram_tensor(f"wv{l}", (128, 4, D), BF16, kind="ExternalInput"),
            "wo": nc.dram_tensor(f"wo{l}", (128, 5, D), BF16, kind="ExternalInput"),
            "wi": nc.dram_tensor(f"wi{l}", (128, 4, F), BF16, kind="ExternalInput"),
            "wo2": nc.dram_tensor(f"wo2{l}", (128, 16, D), BF16, kind="ExternalInput"),
        }
    repr_d = nc.dram_tensor("repr_out", (128, NTC, D), F32, kind="ExternalOutput")
    probs_d = nc.dram_tensor("probs_out", (TOK, NL_RUN, H, L), BF16,
                             kind="ExternalOutput")

    with tile.TileContext(nc) as tc:
        with (
            tc.tile_pool(name="persist", bufs=1) as pp,
            tc.tile_pool(name="work", bufs=1) as wp,
            tc.tile_pool(name="small", bufs=3) as sp,
            tc.tile_pool(name="attn", bufs=4) as ap_pool,
            tc.tile_pool(name="wpool", bufs=2) as wt,
            tc.tile_pool(name="wpool1", bufs=1) as wt1,
            tc.tile_pool(name="psum", bufs=3, space="PSUM") as ps,
            tc.tile_pool(name="psum2", bufs=2, space="PSUM") as ps2,
            tc.tile_pool(name="dram", bufs=2, space="DRAM") as dp,
        ):
            # persistent tiles
            h_sb = pp.tile([128, NTC, D], F32, tag="h")
            cos_sb = pp.tile([128, NTC, CP], BF16, tag="cos")
            sina_sb = pp.tile([128, NTC, 240], BF16, tag="sina")
            sinb_sb = pp.tile([128, NTC, 240], BF16, tag="sinb")
            nc.sync.dma_start(h_sb[:], h0_d.ap())
            nc.sync.dma_start(cos_sb[:], cos_d.ap())
            nc.sync.dma_start(sina_sb[:], sina_d.ap())
            nc.sync.dma_start(sinb_sb[:], sinb_d.ap())

            # padded transpose-source tiles (pad columns stay zero forever)
            x_bf = pp.tile([128, NTC, DP], BF16, tag="xp")
            krot = pp.tile([128, NTC, CP], BF16, tag="krot")
            xown = pp.tile([TOK, DP], BF16, tag="xown")
            qrot = pp.tile([TOK, CP], BF16, tag="qrot")
            x2 = pp.tile([TOK, DP], BF16, tag="x2")
            g_bf = pp.tile([TOK, 4, DP], BF16, tag="g")
            ctxT = pp.tile([128, 5, TOK], BF16, tag="ctxT")
            for t_ in (x_bf, krot, xown, qrot, x2, g_bf, ctxT):
                nc.vector.memset(t_[:], 0.0)

            # own-token data via dynamic DRAM reads (offset = rank*64)
            pid = nc.sync.partition_id()
            tok_off = nc.sync.snap(pid * TOK)

            cos_own = pp.tile([TOK, CP], BF16, tag="cosown")
            sina_own = pp.tile([TOK, 240], BF16, tag="sinaown")
            sinb_own = pp.tile([TOK, 240], BF16, tag="sinbown")
            hown = pp.tile([TOK, D], F32, tag="hown")
            nc.sync.dma_start(cos_own[:], cosf_d.ap()[bass.ds(tok_off, TOK), :])
            nc.sync.dma_start(sina_own[:], sinaf_d.ap()[bass.ds(tok_off, TOK), :])
            nc.sync.dma_start(sinb_own[:], sinbf_d.ap()[bass.ds(tok_off, TOK), :])
            nc.sync.dma_start(hown[:], h0f_d.ap()[bass.ds(tok_off, TOK), :])

            def ln_stats(src2, P, tagp):
                """src2 [P, D] f32 -> (negmean, invstd) [P, 1] tiles."""
                s1 = sp.tile([P, 1], F32, tag=f"{tagp}s1")
                nc.vector.tensor_reduce(s1[:], src2,
                                        axis=mybir.AxisListType.X, op=ALU.add)
                sq = sp.tile([P, D], F32, tag=f"{tagp}sq")
                nc.scalar.activation(sq[:], src2, AF.Square)
                s2 = sp.tile([P, 1], F32, tag=f"{tagp}s2")
                nc.vector.tensor_reduce(s2[:], sq[:],
                                        axis=mybir.AxisListType.X, op=ALU.add)
                negm = sp.tile([P, 1], F32, tag=f"{tagp}m")
                nc.vector.tensor_scalar_mul(negm[:], s1[:], -1.0 / D)
                m2 = sp.tile([P, 1], F32, tag=f"{tagp}m2")
                nc.vector.tensor_tensor(m2[:], negm[:], negm[:], ALU.mult)
                var = sp.tile([P, 1], F32, tag=f"{tagp}v")
                nc.vector.tensor_scalar(var[:], s2[:], 1.0 / D, None, ALU.mult)
                nc.vector.tensor_tensor(var[:], var[:], m2[:], ALU.subtract)
                lnv = sp.tile([P, 1], F32, tag=f"{tagp}lv")
                nc.scalar.activation(lnv[:], var[:], AF.Ln, bias=float(EPS))
                inv = sp.tile([P, 1], F32, tag=f"{tagp}iv")
                nc.scalar.activation(inv[:], lnv[:], AF.Exp, scale=-0.5)
                return negm, inv

            def ln_apply(src2, negm, inv, out_padded):
                nc.vector.tensor_scalar(
                    _real_cols(out_padded),
                    src2.rearrange("p (cc r) -> p cc r", r=120),
                    negm[:], inv[:], ALU.add, ALU.mult)

            def rope(srcs, cos_full, sa, sb, out_full, P, tagp):
                """srcs: list of (psum_ap, col0, ncols) covering [P, CP];
                cos [P, CP], sa/sb [P, 240] as (cc h i); out [P, CP] bf16."""
                for s_ap, c0, nc_ in srcs:
                    nc.vector.tensor_tensor(out_full[:, c0:c0 + nc_], s_ap,
                                            cos_full[:, c0:c0 + nc_], ALU.mult)
                    ncc = nc_ // 128
                    oA, oB = _halves(out_full[:, c0:c0 + nc_], ncc)
                    sA, sB = _halves(s_ap, ncc)
                    i0 = (c0 // 128) * 48
                    sa_v = sa[:, i0:i0 + ncc * 48].rearrange(
                        "p (cc h i) -> p cc h i", h=4, i=12)
                    sb_v = sb[:, i0:i0 + ncc * 48].rearrange(
                        "p (cc h i) -> p cc h i", h=4, i=12)
                    m1 = sp.tile([P, 240], BF16, tag=f"{tagp}m1")
                    m1v = m1[:, 0:ncc * 48].rearrange(
                        "p (cc h i) -> p cc h i", h=4, i=12)
                    nc.vector.tensor_tensor(m1v, sB, sa_v, ALU.mult)
                    nc.vector.tensor_tensor(oA, oA, m1v, ALU.add)
                    m2 = sp.tile([P, 240], BF16, tag=f"{tagp}m2")
                    m2v = m2[:, 0:ncc * 48].rearrange(
                        "p (cc h i) -> p cc h i", h=4, i=12)
                    nc.vector.tensor_tensor(m2v, sA, sb_v, ALU.mult)
                    nc.vector.tensor_tensor(oB, oB, m2v, ALU.add)

            for l in range(NL_RUN):
                wq = wt.tile([128, 4, CP], BF16, tag="wq")
                wk = wt.tile([128, 4, CP], BF16, tag="wk")
                wv = wt.tile([128, 4, D], BF16, tag="wv")
                wo = wt.tile([128, 5, D], BF16, tag="wo")
                wi = wt.tile([128, 4, F], BF16, tag="wi")
                wo2 = wt1.tile([128, 16, D], BF16, tag="wo2")
                for name, t_ in (("wq", wq), ("wk", wk), ("wv", wv),
                                 ("wo", wo), ("wi", wi), ("wo2", wo2)):
                    nc.sync.dma_start(t_[:], w_d[l][name].ap())

                # ---- LN1 (full, replicated) ----
                for t in range(NTC):
                    negm, inv = ln_stats(h_sb[:, t, :], 128, "ln1")
                    ln_apply(h_sb[:, t, :], negm, inv, x_bf[:, t, :])

                # xT: [512 tok, 512 dpad] -> [128, 4, 512]
                xT = wp.tile([128, 4, L], BF16, tag="xT")
                for t in range(NTC):
                    nc.sync.dma_start_transpose(
                        xT[:, :, t * 128:(t + 1) * 128], x_bf[:, t, :])

                # ---- K, V for all tokens (redundant) ----
                v_bf = wp.tile([128, NTC, D], BF16, tag="v")
                for t in range(NTC):
                    kpsA = ps.tile([128, 512], F32, tag="big")
                    kpsB = ps.tile([128, 128], F32, tag="ctx")
                    vps = ps2.tile([128, D], F32, tag="mid")
                    for d in range(4):
                        lhs = xT[:, d, t * 128:(t + 1) * 128]
                        nc.tensor.matmul(kpsA[:], lhs, wk[:, d, 0:512],
                                         start=(d == 0), stop=(d == 3))
                        nc.tensor.matmul(kpsB[:], lhs, wk[:, d, 512:640],
                                         start=(d == 0), stop=(d == 3))
                        nc.tensor.matmul(vps[:], lhs, wv[:, d, :],
                                         start=(d == 0), stop=(d == 3))
                    rope([(kpsA[:], 0, 512), (kpsB[:], 512, 128)],
                         cos_sb[:, t, :], sina_sb[:, t, :],
                         sinb_sb[:, t, :], krot[:, t, :], 128, "rk")
                    nc.scalar.activation(v_bf[:, t, :], vps[:], AF.Copy)

                kT = wp.tile([128, 5, L], BF16, tag="kT")
                for t in range(NTC):
                    nc.sync.dma_start_transpose(
                        kT[:, :, t * 128:(t + 1) * 128], krot[:, t, :])

                # ---- Q for own tokens (own-row LN recomputed locally) ----
                negm, inv = ln_stats(hown[:], TOK, "ln1o")
                ln_apply(hown[:], negm, inv, xown[:])
                xownT = sp.tile([128, 4, TOK], BF16, tag="xownT")
                nc.sync.dma_start_transpose(xownT[:], xown[:])
                qpsA = ps.tile([TOK, 512], F32, tag="big")
                qpsB = ps.tile([TOK, 128], F32, tag="ctx")
                for d in range(4):
                    nc.tensor.matmul(qpsA[:], xownT[:, d, :], wq[:, d, 0:512],
                                     start=(d == 0), stop=(d == 3))
                    nc.tensor.matmul(qpsB[:], xownT[:, d, :], wq[:, d, 512:640],
                                     start=(d == 0), stop=(d == 3))
                rope([(qpsA[:], 0, 512), (qpsB[:], 512, 128)],
                     cos_own[:], sina_own[:], sinb_own[:], qrot[:], TOK, "rq")
                qT = sp.tile([128, 5, TOK], BF16, tag="qT")
                nc.sync.dma_start_transpose(qT[:], qrot[:])

                # ---- attention per head-pair ----
                for hp in range(H // 2):
                    hA, hB = 2 * hp, 2 * hp + 1
                    sc = ps.tile([128, L], F32, tag="big")
                    for j, hh in ((0, hA), (1, hB)):
                        r0 = 32 * (hh % 4)
                        qs = qT[r0:r0 + 24, hh // 4, :]
                        ks = kT[r0:r0 + 24, hh // 4, :]
                        nc.tensor.matmul(sc[64 * j:64 * j + 64, :], qs, ks,
                                         start=True, stop=True,
                                         tile_position=(r0, 64 * j))
                    ex = ap_pool.tile([128, L], BF16, tag="ex")
                    sums = ap_pool.tile([128, 1], F32, tag="sums")
                    nc.scalar.activation(ex[:], sc[:], AF.Exp, accum_out=sums[:])
                    inv2 = ap_pool.tile([128, 1], F32, tag="inv")
                    nc.vector.reciprocal(inv2[:], sums[:])
                    pr = ap_pool.tile([128, L], BF16, tag="pr")
                    nc.vector.tensor_scalar_mul(pr[:], ex[:], inv2[:])
                    nc.sync.dma_start(probs_d.ap()[:, l, hA, :], pr[0:64, :])
                    nc.sync.dma_start(probs_d.ap()[:, l, hB, :], pr[64:128, :])
                    pT = ap_pool.tile([128, 4, 128], BF16, tag="pT")
                    nc.sync.dma_start_transpose(pT[:], pr[:])
                    for j, hh in ((0, hA), (1, hB)):
                        cps = ps.tile([24, TOK], F32, tag="ctx")
                        for kc in range(4):
                            vs = v_bf[:, kc, 24 * hh:24 * hh + 24]
                            nc.tensor.matmul(cps[:], vs,
                                             pT[:, kc, 64 * j:64 * j + 64],
                                             start=(kc == 0), stop=(kc == 3))
                        r0 = 32 * (hh % 4)
                        nc.scalar.activation(ctxT[r0:r0 + 24, hh // 4, :],
                                             cps[:], AF.Copy)

                # ---- O projection (own tokens) ----
                attn_ps = ps2.tile([TOK, D], F32, tag="mid")
                for cc in range(5):
                    nc.tensor.matmul(attn_ps[:], ctxT[:, cc, :], wo[:, cc, :],
                                     start=(cc == 0), stop=(cc == 4))
                h1 = sp.tile([TOK, D], F32, tag="h1")
                nc.vector.tensor_tensor(h1[:], attn_ps[:], hown[:], ALU.add)

                # ---- FFN (own tokens) ----
                negm, inv = ln_stats(h1[:], TOK, "ln2")
                ln_apply(h1[:], negm, inv, x2[:])
                x2T = sp.tile([128, 4, TOK], BF16, tag="x2T")
                nc.sync.dma_start_transpose(x2T[:], x2[:])
                for fc in range(4):
                    ups = ps2.tile([TOK, D], F32, tag="mid")
                    for d in range(4):
                        nc.tensor.matmul(ups[:], x2T[:, d, :],
                                         wi[:, d, fc * D:(fc + 1) * D],
                                         start=(d == 0), stop=(d == 3))
                    nc.scalar.activation(
                        _real_cols(g_bf[:, fc, :]),
                        ups[:].rearrange("p (cc r) -> p cc r", r=120),
                        AF.Gelu)
                gT = wp.tile([128, 16, TOK], BF16, tag="gT")
                gflat = g_bf[:].rearrange("p fc d -> p (fc d)")
                for t in range(4):
                    nc.sync.dma_start_transpose(
                        gT[:, 4 * t:4 * t + 4, :],
                        gflat[:, t * DP:(t + 1) * DP])
                dps = ps2.tile([TOK, D], F32, tag="mid")
                for fi in range(16):
                    nc.tensor.matmul(dps[:], gT[:, fi, :], wo2[:, fi, :],
                                     start=(fi == 0), stop=(fi == 15))

                # ---- delta + AllGather ----
                dsum = sp.tile([TOK, D], F32, tag="dsum")
                nc.vector.tensor_tensor(dsum[:], dps[:], h1[:], ALU.add)
                delta = sp.tile([TOK, D], BF16, tag="delta")
                nc.vector.tensor_tensor(delta[:], dsum[:], hown[:], ALU.subtract)
                ag_in = dp.tile([TOK, D], BF16, tag="agin")
                ag_out = dp.tile([L, D], BF16, tag="agout")
                nc.sync.dma_start(ag_in[:], delta[:])
                nc.gpsimd.collective_compute(
                    "AllGather", ALU.bypass,
                    ins=[ag_in.opt()], outs=[ag_out.opt()],
                    replica_groups=[list(range(N_CORES))],
                )
                dfull = wp.tile([128, NTC, D], BF16, tag="dfull")
                nc.sync.dma_start(
                    dfull[:], ag_out[:].rearrange("(tc p) d -> p tc d", p=128))
                nc.vector.tensor_tensor(h_sb[:], h_sb[:], dfull[:], ALU.add)
                nc.vector.tensor_tensor(hown[:], hown[:], delta[:], ALU.add)

            # ---- final LN (normalized; affine applied host-side) ----
            for t in range(NTC):
                rfin = sp.tile([128, D], F32, tag="rfin")
                negm, inv = ln_stats(h_sb[:, t, :], 128, "lnf")
                nc.vector.tensor_scalar(rfin[:], h_sb[:, t, :],
                                        negm[:], inv[:], ALU.add, ALU.mult)
                nc.sync.dma_start(repr_d.ap()[:, t, :], rfin[:])

    nc.compile()
    return nc


def _pad_rows(a, chunk=120, to=128, nchunks=4):
    """Insert zero rows turning [nchunks*chunk, X] -> [nchunks*to, X]."""
    out = np.zeros((nchunks * to, a.shape[1]), a.dtype)
    for c in range(nchunks):
        out[c * to:c * to + chunk] = a[c * chunk:(c + 1) * chunk]
    return out


def _prep_inputs(inputs):
    ids = np.asarray(inputs["input_ids"]).reshape(L).astype(np.int64)
    mask = np.asarray(inputs["attention_mask"]).reshape(L).astype(np.float32)
    we = np.asarray(inputs["word_emb"], np.float32)

    assert np.all(mask == 1), "kernel assumes all-ones attention mask"

    emb = we[ids].copy()
    is_m = ids == 32
    emb[is_m] = 0.0
    ratio = is_m.astype(np.float32).sum() / mask.sum()
    emb *= (1.0 - 0.15 * 0.8) / (1.0 - ratio)
    emb *= mask[:, None]

    invf = 1.0 / (10000.0 ** (np.arange(0, DH, 2, dtype=np.float32) / DH))
    fr = np.outer(np.arange(L, dtype=np.float32), invf)
    ang = np.concatenate([fr, fr], -1)          # [L, 24]
    cos, sin = np.cos(ang), np.sin(ang)

    # head-slot padded cos pattern [L, 640]: col 128*cc + 32*h' + i
    cosp = np.zeros((L, CP), np.float32)
    for hh in range(H):
        c0 = 128 * (hh // 4) + 32 * (hh % 4)
        cosp[:, c0:c0 + DH] = cos
    sina = np.tile(-sin[:, :12], (1, H))        # [L, 240] == (cc h i)
    sinb = np.tile(sin[:, 12:], (1, H))

    def tchunk(a):   # [L, X] -> [128, NTC, X]
        return np.ascontiguousarray(
            a.reshape(NTC, 128, a.shape[-1]).transpose(1, 0, 2))

    im = {
        "h0": tchunk(emb).astype(np.float32),
        "cos_t": tchunk(cosp).astype(BF),
        "sina_t": tchunk(sina).astype(BF),
        "sinb_t": tchunk(sinb).astype(BF),
        "h0f": emb.astype(np.float32),
        "cosf": cosp.astype(BF),
        "sinaf": sina.astype(BF),
        "sinbf": sinb.astype(BF),
    }

    # column permutation into head-slot padded layout: col 128*cc + 32*h' + i
    cperm = np.zeros((D, CP), np.float32)
    for hh in range(H):
        for i in range(DH):
            cperm[hh * DH + i, 128 * (hh // 4) + 32 * (hh % 4) + i] = 1.0

    for l in range(NL_RUN):
        ln1w = np.asarray(inputs["ln1_w"], np.float32)[l]
        ln2w = np.asarray(inputs["ln2_w"], np.float32)[l]
        for nm in ("bq", "bk", "bv", "bo", "bi", "bo2"):
            assert np.abs(np.asarray(inputs[nm], np.float32)[l]).max() < 1e-30
        assert np.abs(np.asarray(inputs["ln1_b"], np.float32)[l]).max() < 1e-30
        assert np.abs(np.asarray(inputs["ln2_b"], np.float32)[l]).max() < 1e-30

        wq = np.asarray(inputs["Wq"], np.float32)[l] * ln1w[:, None] * (DH ** -0.5)
        wk = np.asarray(inputs["Wk"], np.float32)[l] * ln1w[:, None]
        wv = np.asarray(inputs["Wv"], np.float32)[l] * ln1w[:, None]
        wo = np.asarray(inputs["Wo"], np.float32)[l]
        wi = np.asarray(inputs["Wi"], np.float32)[l] * ln2w[:, None]
        wo2 = np.asarray(inputs["Wo2"], np.float32)[l]

        wq_p = _pad_rows(wq @ cperm)            # [512, 512]
        wk_p = _pad_rows(wk @ cperm)
        wv_p = _pad_rows(wv)                    # [512, 480]
        wo_cp = cperm.T @ wo                    # [640 c-pad rows, 480]
        wi_p = _pad_rows(wi)                    # [512, 1920]
        wo2_p = _pad_rows(wo2, 120, 128, 16)    # [2048, 480]

        im[f"wq{l}"] = np.ascontiguousarray(
            wq_p.reshape(4, 128, CP).transpose(1, 0, 2)).astype(BF)
        im[f"wk{l}"] = np.ascontiguousarray(
            wk_p.reshape(4, 128, CP).transpose(1, 0, 2)).astype(BF)
        im[f"wv{l}"] = np.ascontiguousarray(
            wv_p.reshape(4, 128, D).transpose(1, 0, 2)).astype(BF)
        im[f"wo{l}"] = np.ascontiguousarray(
            wo_cp.reshape(5, 128, D).transpose(1, 0, 2)).astype(BF)
        im[f"wi{l}"] = np.ascontiguousarray(
            wi_p.reshape(4, 128, F).transpose(1, 0, 2)).astype(BF)
        im[f"wo2{l}"] = np.ascontiguousarray(
            wo2_p.reshape(16, 128, D).transpose(1, 0, 2)).astype(BF)
    return im


def kernel(**inputs):
    global LAST_EXEC_NS
    if "nc" not in _CACHED:
        _CACHED["nc"] = build_nc()
    nc = _CACHED["nc"]

    im = _prep_inputs(inputs)
    in_maps = [dict(im) for _ in range(N_CORES)]
    res = run_bass_kernel_spmd(nc, in_maps, core_ids=list(range(N_CORES)))
    LAST_EXEC_NS = res.exec_time_ns

    lnf_w = np.asarray(inputs["lnf_w"], np.float32)
    lnf_b = np.asarray(inputs["lnf_b"], np.float32)
    r = res.results[0]["repr_out"]                       # [128, NTC, D]
    rep = r.transpose(1, 0, 2).reshape(L, D) * lnf_w + lnf_b

    att = np.empty((L, L, NL_RUN * H), np.float32)
    for c in range(N_CORES):
        s = res.results[c]["probs_out"].astype(np.float32)  # [64, NL, H, 512]
        att[c * TOK:(c + 1) * TOK] = (
            s.transpose(0, 3, 1, 2).reshape(TOK, L, NL_RUN * H))
    return (rep.reshape(1, L, D).astype(np.float32),
            att.reshape(1, L, L, NL_RUN * H))


if __name__ == "__main__":
    import reference as R
    inputs = R.setup_inputs()
    out = kernel(**{k: np.asarray(v) for k, v in inputs.items()})
    print("exec_ns:", LAST_EXEC_NS)


# revision 17
# speedup vs baseline: 1.0361x; 1.0361x over previous
"""ESM2 backbone (12-layer dense transformer, D=480, H=20, F=1920, L=512)
on 8 Trainium2 NeuronCores.

Sharding: every core carries the full residual stream h (replicated via a
per-layer bf16 delta AllGather) and redundantly computes LN1 / K / V for all
512 tokens; queries, attention softmax/probs, O-projection and the FFN are
sharded over tokens (64 per core).  The attention-map output is sharded over
the query dim; each core writes its own [64,12,20,512] bf16 scratch which the
host reassembles into the final [1,512,512,240] fp32 tensor.

Layout note: every tensor that feeds a bf16 DMA-transpose or a matmul
contraction is padded from 480 to 512 (8 zero rows/cols inserted after every
120) so the xbar 16x128 tile constraint and full-K matmul chunks line up.

Compute dtype: bf16 matmul operands, fp32 PSUM accumulation, fp32 residual.
"""

import os
import sys

sys.path.insert(0, "/opt/trn_rl_repo")

import numpy as np
import ml_dtypes

import concourse.bass as bass
import concourse.bacc as bacc
import concourse.mybir as mybir
import concourse.tile as tile
from concourse.bass_utils import run_bass_kernel_spmd

BF = ml_dtypes.bfloat16
F32 = mybir.dt.float32
BF16 = mybir.dt.bfloat16
AF = mybir.ActivationFunctionType
ALU = mybir.AluOpType

N_CORES = 8
L, D, H, DH, F, NL = 512, 480, 20, 24, 1920, 12
EPS = 1e-5
TOK = L // N_CORES          # 64 own tokens per core
DP = 512                    # padded feature dim (4 chunks: 120 real + 8 pad)
CP = 640                    # padded q/k head dim (5 chunks: 4 heads x 32 rows)
NTC = L // 128              # 4 token chunks of 128

NL_RUN = int(os.environ.get("ESM_NL_RUN", str(NL)))

LAST_EXEC_NS = None
_CACHED = {}


def _real_cols(ap2, r=120, c=128):
    """[P, n*c] -> [P, n, r] AP of the real (non-pad) columns."""
    return ap2.rearrange("p (cc r) -> p cc r", r=c)[:, :, 0:r]


def _halves(ap2, ncc):
    """[P, ncc*128] head-slot padded -> (halvesA, halvesB) APs [P, ncc, 4, 12]."""
    v = ap2.rearrange("p (cc h s) -> p cc h s", s=32, h=4)
    return v[:, :, :, 0:12], v[:, :, :, 12:24]


def build_nc():
    nc = bacc.Bacc("TRN2", target_bir_lowering=False, debug=False,
                   num_devices=N_CORES)

    eps_t = nc.alloc_sbuf_tensor("const-eps", [128, 1], F32)
    nc.gpsimd.memset(eps_t.ap(), float(EPS))
    nc.const_aps.aps[(F32, float(EPS))] = eps_t.ap()
    nc.all_engine_barrier()

    # ---- DRAM I/O ----
    h0_d = nc.dram_tensor("h0", (128, NTC, D), F32, kind="ExternalInput")
    cos_d = nc.dram_tensor("cos_t", (128, NTC, CP), BF16, kind="ExternalInput")
    sina_d = nc.dram_tensor("sina_t", (128, NTC, 240), BF16, kind="ExternalInput")
    sinb_d = nc.dram_tensor("sinb_t", (128, NTC, 240), BF16, kind="ExternalInput")
    h0f_d = nc.dram_tensor("h0f", (L, D), F32, kind="ExternalInput")
    cosf_d = nc.dram_tensor("cosf", (L, CP), BF16, kind="ExternalInput")
    sinaf_d = nc.dram_tensor("sinaf", (L, 240), BF16, kind="ExternalInput")
    sinbf_d = nc.dram_tensor("sinbf", (L, 240), BF16, kind="ExternalInput")
    w_d = {}
    for l in range(NL_RUN):
        w_d[l] = {
            "wq": nc.dram_tensor(f"wq{l}", (128, 4, CP), BF16, kind="ExternalInput"),
            "wk": nc.dram_tensor(f"wk{l}", (128, 4, CP), BF16, kind="ExternalInput"),
            "wv": nc.dram_tensor(f"wv{l}", (128, 4, D), BF16, kind="ExternalInput"),
            "wo": nc.dram_tensor(f"wo{l}", (128, 5, D), BF16, kind="ExternalInput"),
            "wi": nc.dram_tensor(f"wi{l}", (128, 4, F), BF16, kind="ExternalInput"),
            "wo2": nc.dram_tensor(f"wo2{l}", (128, 16, D), BF16, kind="ExternalInput"),
        }
    repr_d = nc.dram_tensor("repr_out", (128, NTC, D), F32, kind="ExternalOutput")
    probs_d = nc.dram_tensor("probs_out", (TOK, NL_RUN, H, L), BF16,
                             kind="ExternalOutput")

    with tile.TileContext(nc) as tc:
        with (
            tc.tile_pool(name="persist", bufs=1) as pp,
            tc.tile_pool(name="work", bufs=1) as wp,
            tc.tile_pool(name="small", bufs=3) as sp,
            tc.tile_pool(name="attn", bufs=4) as ap_pool,
            tc.tile_pool(name="wpool", bufs=2) as wt,
            tc.tile_pool(name="wpool1", bufs=1) as wt1,
            tc.tile_pool(name="psum", bufs=3, space="PSUM") as ps,
            tc.tile_pool(name="psum2", bufs=2, space="PSUM") as ps2,
            tc.tile_pool(name="dram", bufs=2, space="DRAM") as dp,
        ):
            # persistent tiles
            h_sb = pp.tile([128, NTC, D], F32, tag="h")
            cos_sb = pp.tile([128, NTC, CP], BF16, tag="cos")
            sina_sb = pp.tile([128, NTC, 240], BF16, tag="sina")
            sinb_sb = pp.tile([128, NTC, 240], BF16, tag="sinb")
            nc.sync.dma_start(h_sb[:], h0_d.ap())
            nc.sync.dma_start(cos_sb[:], cos_d.ap())
            nc.sync.dma_start(sina_sb[:], sina_d.ap())
            nc.sync.dma_start(sinb_sb[:], sinb_d.ap())

            # padded transpose-source tiles (pad columns stay zero forever)
            x_bf = pp.tile([128, NTC, DP], BF16, tag="xp")
            krot = pp.tile([128, NTC, CP], BF16, tag="krot")
            xown = pp.tile([TOK, DP], BF16, tag="xown")
            qrot = pp.tile([TOK, CP], BF16, tag="qrot")
            x2 = pp.tile([TOK, DP], BF16, tag="x2")
            g_bf = pp.tile([TOK, 4, DP], BF16, tag="g")
            ctxT = pp.tile([128, 5, TOK], BF16, tag="ctxT")
            for t_ in (x_bf, krot, xown, qrot, x2, g_bf, ctxT):
                nc.vector.memset(t_[:], 0.0)

            # own-token data via dynamic DRAM reads (offset = rank*64)
            pid = nc.sync.partition_id()
            tok_off = nc.sync.snap(pid * TOK)

            cos_own = pp.tile([TOK, CP], BF16, tag="cosown")
            sina_own = pp.tile([TOK, 240], BF16, tag="sinaown")
            sinb_own = pp.tile([TOK, 240], BF16, tag="sinbown")
            hown = pp.tile([TOK, D], F32, tag="hown")
            nc.sync.dma_start(cos_own[:], cosf_d.ap()[bass.ds(tok_off, TOK), :])
            nc.sync.dma_start(sina_own[:], sinaf_d.ap()[bass.ds(tok_off, TOK), :])
            nc.sync.dma_start(sinb_own[:], sinbf_d.ap()[bass.ds(tok_off, TOK), :])
            nc.sync.dma_start(hown[:], h0f_d.ap()[bass.ds(tok_off, TOK), :])

            def ln_stats(src2, P, tagp):
                """src2 [P, D] f32 -> (negmean, invstd) [P, 1] tiles."""
                s1 = sp.tile([P, 1], F32, tag=f"{tagp}s1")
                nc.vector.tensor_reduce(s1[:], src2,
                                        axis=mybir.AxisListType.X, op=ALU.add)
                sq = sp.tile([P, D], F32, tag=f"{tagp}sq")
                nc.scalar.activation(sq[:], src2, AF.Square)
                s2 = sp.tile([P, 1], F32, tag=f"{tagp}s2")
                nc.vector.tensor_reduce(s2[:], sq[:],
                                        axis=mybir.AxisListType.X, op=ALU.add)
                negm = sp.tile([P, 1], F32, tag=f"{tagp}m")
                nc.vector.tensor_scalar_mul(negm[:], s1[:], -1.0 / D)
                m2 = sp.tile([P, 1], F32, tag=f"{tagp}m2")
                nc.vector.tensor_tensor(m2[:], negm[:], negm[:], ALU.mult)
                var = sp.tile([P, 1], F32, tag=f"{tagp}v")
                nc.vector.tensor_scalar(var[:], s2[:], 1.0 / D, None, ALU.mult)
                nc.vector.tensor_tensor(var[:], var[:], m2[:], ALU.subtract)
                lnv = sp.tile([P, 1], F32, tag=f"{tagp}lv")
                nc.scalar.activation(lnv[:], var[:], AF.Ln, bias=float(EPS))
                inv = sp.tile([P, 1], F32, tag=f"{tagp}iv")
                nc.scalar.activation(inv[:], lnv[:], AF.Exp, scale=-0.5)
                return negm, inv

            def ln_apply(src2, negm, inv, out_padded):
                nc.vector.tensor_scalar(
                    _real_cols(out_padded),
                    src2.rearrange("p (cc r) -> p cc r", r=120),
                    negm[:], inv[:], ALU.add, ALU.mult)

            def rope(srcs, cos_full, sa, sb, out_full, P, tagp):
                """srcs: list of (psum_ap, col0, ncols) covering [P, CP];
                cos [P, CP], sa/sb [P, 240] as (cc h i); out [P, CP] bf16."""
                for s_ap, c0, nc_ in srcs:
                    nc.vector.tensor_tensor(out_full[:, c0:c0 + nc_], s_ap,
                                            cos_full[:, c0:c0 + nc_], ALU.mult)
                    ncc = nc_ // 128
                    oA, oB = _halves(out_full[:, c0:c0 + nc_], ncc)
                    sA, sB = _halves(s_ap, ncc)
                    i0 = (c0 // 128) * 48
                    sa_v = sa[:, i0:i0 + ncc * 48].rearrange(
                        "p (cc h i) -> p cc h i", h=4, i=12)
                    sb_v = sb[:, i0:i0 + ncc * 48].rearrange(
                        "p (cc h i) -> p cc h i", h=4, i=12)
                    m1 = sp.tile([P, 240], BF16, tag=f"{tagp}m1")
                    m1v = m1[:, 0:ncc * 48].rearrange(
                        "p (cc h i) -> p cc h i", h=4, i=12)
                    nc.vector.tensor_tensor(m1v, sB, sa_v, ALU.mult)
                    nc.gpsimd.tensor_tensor(oA, oA, m1v, ALU.add)
                    m2 = sp.tile([P, 240], BF16, tag=f"{tagp}m2")
                    m2v = m2[:, 0:ncc * 48].rearrange(
                        "p (cc h i) -> p cc h i", h=4, i=12)
                    nc.vector.tensor_tensor(m2v, sA, sb_v, ALU.mult)
                    nc.gpsimd.tensor_tensor(oB, oB, m2v, ALU.add)

            for l in range(NL_RUN):
                wq = wt.tile([128, 4, CP], BF16, tag="wq")
                wk = wt.tile([128, 4, CP], BF16, tag="wk")
                wv = wt.tile([128, 4, D], BF16, tag="wv")
                wo = wt.tile([128, 5, D], BF16, tag="wo")
                wi = wt.tile([128, 4, F], BF16, tag="wi")
                wo2 = wt1.tile([128, 16, D], BF16, tag="wo2")
                for name, t_ in (("wq", wq), ("wk", wk), ("wv", wv),
                                 ("wo", wo), ("wi", wi), ("wo2", wo2)):
                    nc.scalar.dma_start(t_[:], w_d[l][name].ap())

                # ---- LN1 (full, replicated) ----
                for t in range(NTC):
                    negm, inv = ln_stats(h_sb[:, t, :], 128, "ln1")
                    ln_apply(h_sb[:, t, :], negm, inv, x_bf[:, t, :])

                # xT: [512 tok, 512 dpad] -> [128, 4, 512]
                xT = wp.tile([128, 4, L], BF16, tag="xT")
                for t in range(NTC):
                    nc.sync.dma_start_transpose(
                        xT[:, :, t * 128:(t + 1) * 128], x_bf[:, t, :])

                # ---- K, V for all tokens (redundant) ----
                v_bf = wp.tile([128, NTC, D], BF16, tag="v")
                for t in range(NTC):
                    kpsA = ps.tile([128, 512], F32, tag="big")
                    kpsB = ps.tile([128, 128], F32, tag="ctx")
                    vps = ps2.tile([128, D], F32, tag="mid")
                    for d in range(4):
                        lhs = xT[:, d, t * 128:(t + 1) * 128]
                        nc.tensor.matmul(kpsA[:], lhs, wk[:, d, 0:512],
                                         start=(d == 0), stop=(d == 3))
                        nc.tensor.matmul(kpsB[:], lhs, wk[:, d, 512:640],
                                         start=(d == 0), stop=(d == 3))
                        nc.tensor.matmul(vps[:], lhs, wv[:, d, :],
                                         start=(d == 0), stop=(d == 3))
                    rope([(kpsA[:], 0, 512), (kpsB[:], 512, 128)],
                         cos_sb[:, t, :], sina_sb[:, t, :],
                         sinb_sb[:, t, :], krot[:, t, :], 128, "rk")
                    nc.scalar.activation(v_bf[:, t, :], vps[:], AF.Copy)

                kT = wp.tile([128, 5, L], BF16, tag="kT")
                for t in range(NTC):
                    nc.sync.dma_start_transpose(
                        kT[:, :, t * 128:(t + 1) * 128], krot[:, t, :])

                # ---- Q for own tokens (own-row LN recomputed locally) ----
                negm, inv = ln_stats(hown[:], TOK, "ln1o")
                ln_apply(hown[:], negm, inv, xown[:])
                xownT = sp.tile([128, 4, TOK], BF16, tag="xownT")
                nc.sync.dma_start_transpose(xownT[:], xown[:])
                qpsA = ps.tile([TOK, 512], F32, tag="big")
                qpsB = ps.tile([TOK, 128], F32, tag="ctx")
                for d in range(4):
                    nc.tensor.matmul(qpsA[:], xownT[:, d, :], wq[:, d, 0:512],
                                     start=(d == 0), stop=(d == 3))
                    nc.tensor.matmul(qpsB[:], xownT[:, d, :], wq[:, d, 512:640],
                                     start=(d == 0), stop=(d == 3))
                rope([(qpsA[:], 0, 512), (qpsB[:], 512, 128)],
                     cos_own[:], sina_own[:], sinb_own[:], qrot[:], TOK, "rq")
                qT = sp.tile([128, 5, TOK], BF16, tag="qT")
                nc.sync.dma_start_transpose(qT[:], qrot[:])

                # ---- attention per head-pair ----
                for hp in range(H // 2):
                    hA, hB = 2 * hp, 2 * hp + 1
                    sc = ps.tile([128, L], F32, tag="big")
                    for j, hh in ((0, hA), (1, hB)):
                        r0 = 32 * (hh % 4)
                        qs = qT[r0:r0 + 24, hh // 4, :]
                        ks = kT[r0:r0 + 24, hh // 4, :]
                        nc.tensor.matmul(sc[64 * j:64 * j + 64, :], qs, ks,
                                         start=True, stop=True,
                                         tile_position=(r0, 64 * j))
                    ex = ap_pool.tile([128, L], BF16, tag="ex")
                    sums = ap_pool.tile([128, 1], F32, tag="sums")
                    nc.scalar.activation(ex[:], sc[:], AF.Exp, accum_out=sums[:])
                    inv2 = ap_pool.tile([128, 1], F32, tag="inv")
                    nc.vector.reciprocal(inv2[:], sums[:])
                    pr = ap_pool.tile([128, L], BF16, tag="pr")
                    nc.vector.tensor_scalar_mul(pr[:], ex[:], inv2[:])
                    nc.sync.dma_start(probs_d.ap()[:, l, hA, :], pr[0:64, :])
                    nc.sync.dma_start(probs_d.ap()[:, l, hB, :], pr[64:128, :])
                    pT = ap_pool.tile([128, 4, 128], BF16, tag="pT")
                    nc.sync.dma_start_transpose(pT[:], pr[:])
                    for j, hh in ((0, hA), (1, hB)):
                        cps = ps.tile([24, TOK], F32, tag="ctx")
                        for kc in range(4):
                            vs = v_bf[:, kc, 24 * hh:24 * hh + 24]
                            nc.tensor.matmul(cps[:], vs,
                                             pT[:, kc, 64 * j:64 * j + 64],
                                             start=(kc == 0), stop=(kc == 3))
                        r0 = 32 * (hh % 4)
                        nc.scalar.activation(ctxT[r0:r0 + 24, hh // 4, :],
                                             cps[:], AF.Copy)

                # ---- O projection (own tokens) ----
                attn_ps = ps2.tile([TOK, D], F32, tag="mid")
                for cc in range(5):
                    nc.tensor.matmul(attn_ps[:], ctxT[:, cc, :], wo[:, cc, :],
                                     start=(cc == 0), stop=(cc == 4))
                h1 = sp.tile([TOK, D], F32, tag="h1")
                nc.vector.tensor_tensor(h1[:], attn_ps[:], hown[:], ALU.add)

                # ---- FFN (own tokens) ----
                negm, inv = ln_stats(h1[:], TOK, "ln2")
                ln_apply(h1[:], negm, inv, x2[:])
                x2T = sp.tile([128, 4, TOK], BF16, tag="x2T")
                nc.sync.dma_start_transpose(x2T[:], x2[:])
                for fc in range(4):
                    ups = ps2.tile([TOK, D], F32, tag="mid")
                    for d in range(4):
                        nc.tensor.matmul(ups[:], x2T[:, d, :],
                                         wi[:, d, fc * D:(fc + 1) * D],
                                         start=(d == 0), stop=(d == 3))
                    nc.scalar.activation(
                        _real_cols(g_bf[:, fc, :]),
                        ups[:].rearrange("p (cc r) -> p cc r", r=120),
                        AF.Gelu)
                gT = wp.tile([128, 16, TOK], BF16, tag="gT")
                gflat = g_bf[:].rearrange("p fc d -> p (fc d)")
                for t in range(4):
                    nc.sync.dma_start_transpose(
                        gT[:, 4 * t:4 * t + 4, :],
                        gflat[:, t * DP:(t + 1) * DP])
                dps = ps2.tile([TOK, D], F32, tag="mid")
                for fi in range(16):
                    nc.tensor.matmul(dps[:], gT[:, fi, :], wo2[:, fi, :],
                                     start=(fi == 0), stop=(fi == 15))

                # ---- delta + AllGather ----
                dsum = sp.tile([TOK, D], F32, tag="dsum")
                nc.vector.tensor_tensor(dsum[:], dps[:], h1[:], ALU.add)
                delta = sp.tile([TOK, D], BF16, tag="delta")
                nc.vector.tensor_tensor(delta[:], dsum[:], hown[:], ALU.subtract)
                ag_in = dp.tile([TOK, D], BF16, tag="agin")
                ag_out = dp.tile([L, D], BF16, tag="agout")
                nc.sync.dma_start(ag_in[:], delta[:])
                nc.gpsimd.collective_compute(
                    "AllGather", ALU.bypass,
                    ins=[ag_in.opt()], outs=[ag_out.opt()],
                    replica_groups=[list(range(N_CORES))],
                )
                dfull = wp.tile([128, NTC, D], BF16, tag="dfull")
                nc.sync.dma_start(
                    dfull[:], ag_out[:].rearrange("(tc p) d -> p tc d", p=128))
                for t in range(NTC):
                    nc.vector.tensor_tensor(h_sb[:, t, :], h_sb[:, t, :],
                                            dfull[:, t, :], ALU.add)
                nc.vector.tensor_tensor(hown[:], hown[:], delta[:], ALU.add)

            # ---- final LN (normalized; affine applied host-side) ----
            for t in range(NTC):
                rfin = sp.tile([128, D], F32, tag="rfin")
                negm, inv = ln_stats(h_sb[:, t, :], 128, "lnf")
                nc.vector.tensor_scalar(rfin[:], h_sb[:, t, :],
                                        negm[:], inv[:], ALU.add, ALU.mult)
                nc.sync.dma_start(repr_d.ap()[:, t, :], rfin[:])

    nc.compile()
    return nc


def _pad_rows(a, chunk=120, to=128, nchunks=4):
    """Insert zero rows turning [nchunks*chunk, X] -> [nchunks*to, X]."""
    out = np.zeros((nchunks * to, a.shape[1]), a.dtype)
    for c in range(nchunks):
        out[c * to:c * to + chunk] = a[c * chunk:(c + 1) * chunk]
    return out


def _prep_inputs(inputs):
    ids = np.asarray(inputs["input_ids"]).reshape(L).astype(np.int64)
    mask = np.asarray(inputs["attention_mask"]).reshape(L).astype(np.float32)
    we = np.asarray(inputs["word_emb"], np.float32)

    assert np.all(mask == 1), "kernel assumes all-ones attention mask"

    emb = we[ids].copy()
    is_m = ids == 32
    emb[is_m] = 0.0
    ratio = is_m.astype(np.float32).sum() / mask.sum()
    emb *= (1.0 - 0.15 * 0.8) / (1.0 - ratio)
    emb *= mask[:, None]

    invf = 1.0 / (10000.0 ** (np.arange(0, DH, 2, dtype=np.float32) / DH))
    fr = np.outer(np.arange(L, dtype=np.float32), invf)
    ang = np.concatenate([fr, fr], -1)          # [L, 24]
    cos, sin = np.cos(ang), np.sin(ang)

    # head-slot padded cos pattern [L, 640]: col 128*cc + 32*h' + i
    cosp = np.zeros((L, CP), np.float32)
    for hh in range(H):
        c0 = 128 * (hh // 4) + 32 * (hh % 4)
        cosp[:, c0:c0 + DH] = cos
    sina = np.tile(-sin[:, :12], (1, H))        # [L, 240] == (cc h i)
    sinb = np.tile(sin[:, 12:], (1, H))

    def tchunk(a):   # [L, X] -> [128, NTC, X]
        return np.ascontiguousarray(
            a.reshape(NTC, 128, a.shape[-1]).transpose(1, 0, 2))

    im = {
        "h0": tchunk(emb).astype(np.float32),
        "cos_t": tchunk(cosp).astype(BF),
        "sina_t": tchunk(sina).astype(BF),
        "sinb_t": tchunk(sinb).astype(BF),
        "h0f": emb.astype(np.float32),
        "cosf": cosp.astype(BF),
        "sinaf": sina.astype(BF),
        "sinbf": sinb.astype(BF),
    }

    # column permutation into head-slot padded layout: col 128*cc + 32*h' + i
    cperm = np.zeros((D, CP), np.float32)
    for hh in range(H):
        for i in range(DH):
            cperm[hh * DH + i, 128 * (hh // 4) + 32 * (hh % 4) + i] = 1.0

    for l in range(NL_RUN):
        ln1w = np.asarray(inputs["ln1_w"], np.float32)[l]
        ln2w = np.asarray(inputs["ln2_w"], np.float32)[l]
        for nm in ("bq", "bk", "bv", "bo", "bi", "bo2"):
            assert np.abs(np.asarray(inputs[nm], np.float32)[l]).max() < 1e-30
        assert np.abs(np.asarray(inputs["ln1_b"], np.float32)[l]).max() < 1e-30
        assert np.abs(np.asarray(inputs["ln2_b"], np.float32)[l]).max() < 1e-30

        wq = np.asarray(inputs["Wq"], np.float32)[l] * ln1w[:, None] * (DH ** -0.5)
        wk = np.asarray(inputs["Wk"], np.float32)[l] * ln1w[:, None]
        wv = np.asarray(inputs["Wv"], np.float32)[l] * ln1w[:, None]
        wo = np.asarray(inputs["Wo"], np.float32)[l]
        wi = np.asarray(inputs["Wi"], np.float32)[l] * ln2w[:, None]
        wo2 = np.asarray(inputs["Wo2"], np.float32)[l]

        wq_p = _pad_rows(wq @ cperm)            # [512, 512]
        wk_p = _pad_rows(wk @ cperm)
        wv_p = _pad_rows(wv)                    # [512, 480]
        wo_cp = cperm.T @ wo                    # [640 c-pad rows, 480]
        wi_p = _pad_rows(wi)                    # [512, 1920]
        wo2_p = _pad_rows(wo2, 120, 128, 16)    # [2048, 480]

        im[f"wq{l}"] = np.ascontiguousarray(
            wq_p.reshape(4, 128, CP).transpose(1, 0, 2)).astype(BF)
        im[f"wk{l}"] = np.ascontiguousarray(
            wk_p.reshape(4, 128, CP).transpose(1, 0, 2)).astype(BF)
        im[f"wv{l}"] = np.ascontiguousarray(
            wv_p.reshape(4, 128, D).transpose(1, 0, 2)).astype(BF)
        im[f"wo{l}"] = np.ascontiguousarray(
            wo_cp.reshape(5, 128, D).transpose(1, 0, 2)).astype(BF)
        im[f"wi{l}"] = np.ascontiguousarray(
            wi_p.reshape(4, 128, F).transpose(1, 0, 2)).astype(BF)
        im[f"wo2{l}"] = np.ascontiguousarray(
            wo2_p.reshape(16, 128, D).transpose(1, 0, 2)).astype(BF)
    return im


def kernel(**inputs):
    global LAST_EXEC_NS
    if "nc" not in _CACHED:
        _CACHED["nc"] = build_nc()
    nc = _CACHED["nc"]

    im = _prep_inputs(inputs)
    in_maps = [dict(im) for _ in range(N_CORES)]
    res = run_bass_kernel_spmd(nc, in_maps, core_ids=list(range(N_CORES)))
    LAST_EXEC_NS = res.exec_time_ns

    lnf_w = np.asarray(inputs["lnf_w"], np.float32)
    lnf_b = np.asarray(inputs["lnf_b"], np.float32)
    r = res.results[0]["repr_out"]                       # [128, NTC, D]
    rep = r.transpose(1, 0, 2).reshape(L, D) * lnf_w + lnf_b

    att = np.empty((L, L, NL_RUN * H), np.float32)
    for c in range(N_CORES):
        s = res.results[c]["probs_out"].astype(np.float32)  # [64, NL, H, 512]
        att[c * TOK:(c + 1) * TOK] = (
            s.transpose(0, 3, 1, 2).reshape(TOK, L, NL_RUN * H))
    return (rep.reshape(1, L, D).astype(np.float32),
            att.reshape(1, L, L, NL_RUN * H))


if __name__ == "__main__":
    import reference as R
    inputs = R.setup_inputs()
    out = kernel(**{k: np.asarray(v) for k, v in inputs.items()})
    print("exec_ns:", LAST_EXEC_NS)
